# revision 15
# baseline (speedup 1.0000x reference)
# Additive (Bahdanau) attention Trainium2 kernel.
#
# Problem shapes (hardcoded): B=4, Tq=256, Tv=1024, D=512, A=128.
#   k = inputs @ Wk + bk                  [B,Tv,A]
#   q = context @ Wq + bq                 [B,Tq,A]
#   scores[b,i,v] = sum_a attn_v[a] * tanh(q[b,i,a] + k[b,v,a]) + (1-mask)*NEG_BIG
#   out = softmax_v(scores) @ inputs      [B,Tq,D]
#
# Sharding: 8 cores = (batch b = c//2) x (query half qh = c%2); each core owns
# 128 queries with the full Tv, so softmax is local and no collectives are
# needed.
#
# Per-core dataflow (ACT/tanh-bound):
#   PE:  transpose inputs/context -> kT[a,v] (PSUM->SBUF), qT[a,q] projections
#   DVE: S[a, (j,v)] = kT[a,v] + qb[a, q]      (tensor_scalar, 2x mode)
#   ACT: T = tanh(S) on G-query batches        (the 16.8M-element bottleneck)
#   PE:  scores[q,v] accumulated with shifted one-hot weight columns so each
#        query's weighted A-reduction lands on its own PSUM partition
#   softmax: DVE reduce_max(negate) -> ACT exp(bias=-max, accum_out=sumexp)
#   PE:  transpose exp(P) -> P^T; out = P^T.T @ inputs; scale by 1/sumexp

import numpy as np

import concourse.bass as bass
import concourse.tile as tile
from concourse import bacc, mybir
from concourse import bass_utils
from concourse.masks import make_identity

P = 128
B, Tq, Tv, D, A = 4, 256, 1024, 512, 128
NCORES = 8
QC = Tq // 2          # queries per core
DC = D // P           # d chunks (4)
VB = Tv // P          # v blocks (8)
G = 4                 # queries per tanh batch
NG = QC // G          # groups (32)
NEG_BIG = -1e9

F32 = mybir.dt.float32
F32R = mybir.dt.float32r
I32 = mybir.dt.int32
AF = mybir.ActivationFunctionType


def _r(ap):
    # fp32 matmuls stream at 4 cycles/row on the PE; float32r (same bytes,
    # reduced-precision multiply, fp32 accumulate) streams at 1 cycle/row
    # for free dims >= 256.
    return ap.bitcast(F32R)


def build_nc():
    nc = bacc.Bacc("TRN2", target_bir_lowering=False, debug=False)

    inp_d = nc.dram_tensor("inp", (Tv, D), F32R, kind="ExternalInput")
    ctx_d = nc.dram_tensor("ctx", (QC, D), F32R, kind="ExternalInput")
    msk_d = nc.dram_tensor("mask", (1, Tv), I32, kind="ExternalInput")
    wkq_d = nc.dram_tensor("wkq", (D, 2 * A), F32R, kind="ExternalInput")
    bba_d = nc.dram_tensor("bba", (A, 3), F32, kind="ExternalInput")
    y_d = nc.dram_tensor("y", (QC, D), F32, kind="ExternalOutput")

    with tile.TileContext(nc) as tc:
        with (
            tc.tile_pool(name="const", bufs=1) as const,
            tc.tile_pool(name="spool", bufs=3) as spool,
            tc.tile_pool(name="tpool", bufs=3) as tpool,
            tc.tile_pool(name="ps_tr", bufs=2, space="PSUM") as ps_tr,
            tc.tile_pool(name="ps_proj", bufs=2, space="PSUM") as ps_proj,
            tc.tile_pool(name="ps_sc", bufs=1, space="PSUM") as ps_sc,
        ):
            # ---- loads (DMA count minimized: ~650ns HWDGE issue each) ----
            ctx_sb = const.tile([P, D], F32R)
            nc.sync.dma_start(ctx_sb[:], ctx_d.ap())
            wkq_sb = const.tile([P, DC, 2 * A], F32R)
            nc.sync.dma_start(wkq_sb[:], wkq_d.ap().rearrange("(o p) a -> p o a", p=P))
            inp_re = inp_d.ap().rearrange("(o p) d -> p o d", p=P)
            inp_h = [const.tile([P, 4, D], F32R, name=f"inph{h}") for h in range(2)]
            nc.sync.dma_start(inp_h[0][:], inp_re[:, 0:4, :])
            bba_sb = const.tile([P, 3], F32)
            nc.sync.dma_start(bba_sb[:], bba_d.ap())
            msk_sb = const.tile([1, Tv], I32)
            nc.sync.dma_start(msk_sb[:], msk_d.ap())
            nc.sync.dma_start(inp_h[1][:], inp_re[:, 4:8, :])
            bk_sb = bba_sb[:, 0:1]
            bq_sb = bba_sb[:, 1:2]
            av_sb = bba_sb[:, 2:3]
            wk_sb = wkq_sb[:, :, 0:A]
            wq_sb = wkq_sb[:, :, A : 2 * A]

            ident = const.tile([P, P], F32)
            make_identity(nc, ident[:])
            ident_r = const.tile([P, P], F32R)
            nc.vector.tensor_copy(ident_r[:], ident[:])

            # mask -> additive row: neg[v] = mask*1e9 - 1e9  (0 if mask==1)
            mskf_sb = const.tile([1, Tv], F32)
            nc.vector.tensor_copy(mskf_sb[:], msk_sb[:])
            neg_sb = const.tile([1, Tv], F32R)
            nc.vector.tensor_scalar(
                neg_sb[:], mskf_sb[:], -NEG_BIG, NEG_BIG,
                mybir.AluOpType.mult, mybir.AluOpType.add,
            )
            stage = const.tile([P, 2 * P - 1], F32)
            nc.vector.memset(stage[:], 0.0)
            ones1 = const.tile([1, P], F32R)
            onesf = const.tile([1, P], F32)
            nc.vector.memset(onesf[:], 1.0)
            nc.vector.tensor_copy(ones1[:], onesf[:])

            # shifted one-hot weights: BIGT[:, 127] = attn_v, else 0
            bigt = const.tile([P, 2 * P - 1], F32R)
            nc.vector.tensor_copy(bigt[:], stage[:])
            nc.vector.tensor_copy(bigt[:, P - 1 : P], av_sb[:])

            # ---- transposes: context -> ctxT [d, q], inputs -> inputsT [d, v] ----
            ctxT_sb = const.tile([P, DC, P], F32R)
            trc = ps_tr.tile([P, 512], F32R, tag="tr_r")
            for dc in range(DC):
                nc.tensor.transpose(
                    trc[:, dc * P : (dc + 1) * P],
                    ctx_sb[:, dc * P : (dc + 1) * P],
                    ident_r[:],
                )
            nc.any.tensor_copy(ctxT_sb[:], trc[:])

            inpT_sb = const.tile([P, DC, Tv], F32R)
            for vb in range(VB):
                trv = ps_tr.tile([P, 512], F32R, tag="tr_r")
                for dc in range(DC):
                    nc.tensor.transpose(
                        trv[:, dc * P : (dc + 1) * P],
                        inp_h[vb // 4][:, vb % 4, dc * P : (dc + 1) * P],
                        ident_r[:],
                    )
                nc.any.tensor_copy(
                    inpT_sb[:, :, vb * P : (vb + 1) * P], trv[:]
                )

            # ---- projections ----
            # kT[a, v] = sum_d Wk[d,a] * inputsT[d,v]
            kT_sb = const.tile([P, Tv], F32)
            for h in range(2):
                pk = ps_proj.tile([P, 512], F32, tag="proj")
                for dc in range(DC):
                    nc.tensor.matmul(
                        pk[:],
                        wk_sb[:, dc, :],
                        inpT_sb[:, dc, h * 512 : (h + 1) * 512],
                        start=(dc == 0),
                        stop=(dc == DC - 1),
                    )
                nc.any.tensor_copy(kT_sb[:, h * 512 : (h + 1) * 512], pk[:])

            # qb[a, q] = sum_d Wq[d,a] * ctxT[d,q] + (bk+bq)[a]
            bkq_sb = const.tile([P, 1], F32)
            nc.vector.tensor_add(bkq_sb[:], bk_sb[:], bq_sb[:])
            pq = ps_proj.tile([P, P], F32, tag="proj")
            for dc in range(DC):
                nc.tensor.matmul(
                    pq[:],
                    wq_sb[:, dc, :],
                    ctxT_sb[:, dc, :],
                    start=(dc == 0),
                    stop=(dc == DC - 1),
                )
            qb_sb = const.tile([P, P], F32)
            nc.vector.tensor_scalar_add(qb_sb[:], pq[:], bkq_sb[:])

            # ---- main loop: tanh batches + one-hot score reduction ----
            scores = ps_sc.tile([P, Tv], F32)
            for g in range(NG):
                s_t = spool.tile([P, G, Tv], F32, tag="S")
                for j in range(G):
                    if g < 3:
                        # split by v-half so the pipeline can start as soon as
                        # the first half of kT (first 4 input blocks) is ready
                        for h in range(2):
                            nc.vector.tensor_scalar_add(
                                s_t[:, j, h * 512 : (h + 1) * 512],
                                kT_sb[:, h * 512 : (h + 1) * 512],
                                qb_sb[:, g * G + j : g * G + j + 1],
                            )
                    else:
                        nc.vector.tensor_scalar_add(
                            s_t[:, j, :], kT_sb[:], qb_sb[:, g * G + j : g * G + j + 1]
                        )
                t_t = tpool.tile([P, G, Tv], F32R, tag="T")
                if g < 3:
                    for h in range(2):
                        nc.scalar.activation(
                            t_t[:, :, h * 512 : (h + 1) * 512],
                            s_t[:, :, h * 512 : (h + 1) * 512],
                            AF.Tanh,
                        )
                else:
                    nc.scalar.activation(t_t[:], s_t[:], AF.Tanh)
                for j in range(G):
                    q = g * G + j
                    for h in range(2):
                        nc.tensor.matmul(
                            scores[:, h * 512 : (h + 1) * 512],
                            bigt[:, P - 1 - q : 2 * P - 1 - q],
                            t_t[:, j, h * 512 : (h + 1) * 512],
                            start=(q == 0),
                            stop=False,
                        )
            # additive mask row broadcast to all query partitions (rank-1)
            for h in range(2):
                nc.tensor.matmul(
                    scores[:, h * 512 : (h + 1) * 512],
                    ones1[:],
                    neg_sb[:, h * 512 : (h + 1) * 512],
                    start=False,
                    stop=True,
                )

            # ---- softmax over v (free dim); scores are bounded by
            # ||attn_v||_1 (~9.2 for this problem scale), so raw exp is safe
            # in fp32 and the max-subtraction can be skipped ----
            expP = const.tile([P, Tv], F32R)
            sumexp_h = const.tile([P, 2], F32)
            for h in range(2):
                nc.scalar.activation(
                    expP[:, h * 512 : (h + 1) * 512],
                    scores[:, h * 512 : (h + 1) * 512],
                    AF.Exp,
                    accum_out=sumexp_h[:, h : h + 1],
                )
            sumexp = const.tile([P, 1], F32)
            nc.vector.tensor_reduce(
                sumexp[:], sumexp_h[:], axis=mybir.AxisListType.X,
                op=mybir.AluOpType.add,
            )
            recip = const.tile([P, 1], F32)
            nc.vector.reciprocal(recip[:], sumexp[:])

            # ---- P^T, final matmul, scale ----
            pT_sb = const.tile([P, VB, P], F32R)
            for half in range(2):
                trp = ps_tr.tile([P, 512], F32R, tag="tr_r")
                for i in range(4):
                    vb = half * 4 + i
                    nc.tensor.transpose(
                        trp[:, i * P : (i + 1) * P],
                        expP[:, vb * P : (vb + 1) * P],
                        ident_r[:],
                    )
                nc.any.tensor_copy(pT_sb[:, half * 4 : (half + 1) * 4, :], trp[:])

            po = ps_proj.tile([P, 512], F32, tag="proj")
            for vb in range(VB):
                nc.tensor.matmul(
                    po[:],
                    pT_sb[:, vb, :],
                    inp_h[vb // 4][:, vb % 4, :],
                    start=(vb == 0),
                    stop=(vb == VB - 1),
                )
            out_sb = const.tile([P, D], F32)
            nc.vector.tensor_scalar_mul(out_sb[:], po[:], recip[:])
            nc.sync.dma_start(y_d.ap(), out_sb[:])

    nc.compile()
    return nc


_NC_CACHE = None


def _get_nc():
    global _NC_CACHE
    if _NC_CACHE is None:
        _NC_CACHE = build_nc()
    return _NC_CACHE


def kernel(inputs, context, mask, Wk, bk, Wq, bq, attn_v):
    nc = _get_nc()
    f32 = np.float32
    in_maps = []
    for c in range(NCORES):
        b, qh = c // 2, c % 2
        wkq = np.concatenate(
            [np.asarray(Wk, dtype=f32), np.asarray(Wq, dtype=f32)], axis=1
        )
        bba = np.stack(
            [np.asarray(bk, f32), np.asarray(bq, f32), np.asarray(attn_v, f32)],
            axis=1,
        )
        in_maps.append({
            "inp": np.ascontiguousarray(inputs[b], dtype=f32),
            "ctx": np.ascontiguousarray(
                context[b, qh * QC : (qh + 1) * QC], dtype=f32
            ),
            "mask": np.ascontiguousarray(mask[b : b + 1, :], dtype=np.int32),
            "wkq": np.ascontiguousarray(wkq),
            "bba": np.ascontiguousarray(bba),
        })
    res = bass_utils.run_bass_kernel_spmd(nc, in_maps, core_ids=list(range(NCORES)))
    out = np.empty((B, Tq, D), f32)
    for c in range(NCORES):
        b, qh = c // 2, c % 2
        out[b, qh * QC : (qh + 1) * QC, :] = res.results[c]["y"]
    return out


# revision 17
# speedup vs baseline: 1.0067x; 1.0067x over previous
# Additive (Bahdanau) attention Trainium2 kernel.
#
# Problem shapes (hardcoded): B=4, Tq=256, Tv=1024, D=512, A=128.
#   k = inputs @ Wk + bk                  [B,Tv,A]
#   q = context @ Wq + bq                 [B,Tq,A]
#   scores[b,i,v] = sum_a attn_v[a] * tanh(q[b,i,a] + k[b,v,a]) + (1-mask)*NEG_BIG
#   out = softmax_v(scores) @ inputs      [B,Tq,D]
#
# Sharding: 8 cores = (batch b = c//2) x (query half qh = c%2); each core owns
# 128 queries with the full Tv, so softmax is local and no collectives are
# needed.
#
# Per-core dataflow (ACT/tanh-bound):
#   PE:  transpose inputs/context -> kT[a,v] (PSUM->SBUF), qT[a,q] projections
#   DVE: S[a, (j,v)] = kT[a,v] + qb[a, q]      (tensor_scalar, 2x mode)
#   ACT: T = tanh(S) on G-query batches        (the 16.8M-element bottleneck)
#   PE:  scores[q,v] accumulated with shifted one-hot weight columns so each
#        query's weighted A-reduction lands on its own PSUM partition
#   softmax: DVE reduce_max(negate) -> ACT exp(bias=-max, accum_out=sumexp)
#   PE:  transpose exp(P) -> P^T; out = P^T.T @ inputs; scale by 1/sumexp

import numpy as np

import concourse.bass as bass
import concourse.tile as tile
from concourse import bacc, mybir
from concourse import bass_utils
from concourse.masks import make_identity

P = 128
B, Tq, Tv, D, A = 4, 256, 1024, 512, 128
NCORES = 8
QC = Tq // 2          # queries per core
DC = D // P           # d chunks (4)
VB = Tv // P          # v blocks (8)
G = 4                 # queries per tanh batch
NG = QC // G          # groups (32)
NEG_BIG = -1e9

F32 = mybir.dt.float32
F32R = mybir.dt.float32r
I32 = mybir.dt.int32
AF = mybir.ActivationFunctionType


def _r(ap):
    # fp32 matmuls stream at 4 cycles/row on the PE; float32r (same bytes,
    # reduced-precision multiply, fp32 accumulate) streams at 1 cycle/row
    # for free dims >= 256.
    return ap.bitcast(F32R)


def build_nc():
    nc = bacc.Bacc("TRN2", target_bir_lowering=False, debug=False)

    inp_d = nc.dram_tensor("inp", (Tv, D), F32R, kind="ExternalInput")
    ctx_d = nc.dram_tensor("ctx", (QC, D), F32R, kind="ExternalInput")
    msk_d = nc.dram_tensor("mask", (1, Tv), I32, kind="ExternalInput")
    wkq_d = nc.dram_tensor("wkq", (D, 2 * A), F32R, kind="ExternalInput")
    bba_d = nc.dram_tensor("bba", (A, 3), F32, kind="ExternalInput")
    y_d = nc.dram_tensor("y", (QC, D), F32, kind="ExternalOutput")

    with tile.TileContext(nc) as tc:
        with (
            tc.tile_pool(name="const", bufs=1) as const,
            tc.tile_pool(name="spool", bufs=3) as spool,
            tc.tile_pool(name="tpool", bufs=3) as tpool,
            tc.tile_pool(name="ps_tr", bufs=2, space="PSUM") as ps_tr,
            tc.tile_pool(name="ps_proj", bufs=2, space="PSUM") as ps_proj,
            tc.tile_pool(name="ps_sc", bufs=1, space="PSUM") as ps_sc,
        ):
            # ---- loads (DMA count minimized: ~650ns HWDGE issue each) ----
            ctx_sb = const.tile([P, D], F32R)
            nc.sync.dma_start(ctx_sb[:], ctx_d.ap())
            wkq_sb = const.tile([P, DC, 2 * A], F32R)
            nc.sync.dma_start(wkq_sb[:], wkq_d.ap().rearrange("(o p) a -> p o a", p=P))
            inp_re = inp_d.ap().rearrange("(o p) d -> p o d", p=P)
            inp_h = [const.tile([P, 4, D], F32R, name=f"inph{h}") for h in range(2)]
            nc.sync.dma_start(inp_h[0][:], inp_re[:, 0:4, :])
            bba_sb = const.tile([P, 3], F32)
            nc.sync.dma_start(bba_sb[:], bba_d.ap())
            msk_sb = const.tile([1, Tv], I32)
            nc.sync.dma_start(msk_sb[:], msk_d.ap())
            nc.sync.dma_start(inp_h[1][:], inp_re[:, 4:8, :])
            bk_sb = bba_sb[:, 0:1]
            bq_sb = bba_sb[:, 1:2]
            av_sb = bba_sb[:, 2:3]
            wk_sb = wkq_sb[:, :, 0:A]
            wq_sb = wkq_sb[:, :, A : 2 * A]

            ident = const.tile([P, P], F32)
            make_identity(nc, ident[:])
            ident_r = const.tile([P, P], F32R)
            nc.vector.tensor_copy(ident_r[:], ident[:])

            # mask -> additive row: neg[v] = mask*1e9 - 1e9  (0 if mask==1)
            mskf_sb = const.tile([1, Tv], F32)
            nc.vector.tensor_copy(mskf_sb[:], msk_sb[:])
            neg_sb = const.tile([1, Tv], F32R)
            nc.vector.tensor_scalar(
                neg_sb[:], mskf_sb[:], -NEG_BIG, NEG_BIG,
                mybir.AluOpType.mult, mybir.AluOpType.add,
            )
            stage = const.tile([P, 2 * P - 1], F32)
            nc.vector.memset(stage[:], 0.0)
            ones1 = const.tile([1, P], F32R)
            onesf = const.tile([1, P], F32)
            nc.vector.memset(onesf[:], 1.0)
            nc.vector.tensor_copy(ones1[:], onesf[:])

            # shifted one-hot weights: BIGT[:, 127] = attn_v, else 0
            bigt = const.tile([P, 2 * P - 1], F32R)
            nc.vector.tensor_copy(bigt[:], stage[:])
            nc.vector.tensor_copy(bigt[:, P - 1 : P], av_sb[:])

            # ---- transposes: context -> ctxT [d, q], inputs -> inputsT [d, v] ----
            ctxT_sb = const.tile([P, DC, P], F32R)
            trc = ps_tr.tile([P, 512], F32R, tag="tr_r")
            for dc in range(DC):
                nc.tensor.transpose(
                    trc[:, dc * P : (dc + 1) * P],
                    ctx_sb[:, dc * P : (dc + 1) * P],
                    ident_r[:],
                )
            nc.any.tensor_copy(ctxT_sb[:], trc[:])

            inpT_sb = const.tile([P, DC, Tv], F32R)
            for vb in range(VB):
                trv = ps_tr.tile([P, 512], F32R, tag="tr_r")
                for dc in range(DC):
                    nc.tensor.transpose(
                        trv[:, dc * P : (dc + 1) * P],
                        inp_h[vb // 4][:, vb % 4, dc * P : (dc + 1) * P],
                        ident_r[:],
                    )
                nc.any.tensor_copy(
                    inpT_sb[:, :, vb * P : (vb + 1) * P], trv[:]
                )

            # ---- projections ----
            # kT[a, v] = sum_d Wk[d,a] * inputsT[d,v]
            kT_sb = const.tile([P, Tv], F32)
            for h in range(2):
                pk = ps_proj.tile([P, 512], F32, tag="proj")
                for dc in range(DC):
                    nc.tensor.matmul(
                        pk[:],
                        wk_sb[:, dc, :],
                        inpT_sb[:, dc, h * 512 : (h + 1) * 512],
                        start=(dc == 0),
                        stop=(dc == DC - 1),
                    )
                nc.any.tensor_copy(kT_sb[:, h * 512 : (h + 1) * 512], pk[:])

            # qb[a, q] = sum_d Wq[d,a] * ctxT[d,q] + (bk+bq)[a]
            bkq_sb = const.tile([P, 1], F32)
            nc.vector.tensor_add(bkq_sb[:], bk_sb[:], bq_sb[:])
            pq = ps_proj.tile([P, P], F32, tag="proj")
            for dc in range(DC):
                nc.tensor.matmul(
                    pq[:],
                    wq_sb[:, dc, :],
                    ctxT_sb[:, dc, :],
                    start=(dc == 0),
                    stop=(dc == DC - 1),
                )
            qb_sb = const.tile([P, P], F32)
            nc.vector.tensor_scalar_add(qb_sb[:], pq[:], bkq_sb[:])

            # ---- main loop: tanh batches + one-hot score reduction ----
            scores = ps_sc.tile([P, Tv], F32)
            # First NPRE groups run h-major: all half-0 work is emitted before
            # any half-1 work, so the in-order DVE/ACT streams never block on
            # the second half of kT (which waits on the second input DMA).
            NPRE = 3
            s_pre = [spool.tile([P, G, Tv], F32, tag="S", name=f"s_pre{i}") for i in range(NPRE)]
            t_pre = [tpool.tile([P, G, Tv], F32R, tag="T", name=f"t_pre{i}") for i in range(NPRE)]
            for h in range(2):
                for g in range(NPRE):
                    for j in range(G):
                        nc.vector.tensor_scalar_add(
                            s_pre[g][:, j, h * 512 : (h + 1) * 512],
                            kT_sb[:, h * 512 : (h + 1) * 512],
                            qb_sb[:, g * G + j : g * G + j + 1],
                        )
                    nc.scalar.activation(
                        t_pre[g][:, :, h * 512 : (h + 1) * 512],
                        s_pre[g][:, :, h * 512 : (h + 1) * 512],
                        AF.Tanh,
                    )
                    for j in range(G):
                        q = g * G + j
                        nc.tensor.matmul(
                            scores[:, h * 512 : (h + 1) * 512],
                            bigt[:, P - 1 - q : 2 * P - 1 - q],
                            t_pre[g][:, j, h * 512 : (h + 1) * 512],
                            start=(q == 0),
                            stop=False,
                        )
            for g in range(NPRE, NG):
                s_t = spool.tile([P, G, Tv], F32, tag="S")
                for j in range(G):
                    nc.vector.tensor_scalar_add(
                        s_t[:, j, :], kT_sb[:], qb_sb[:, g * G + j : g * G + j + 1]
                    )
                t_t = tpool.tile([P, G, Tv], F32R, tag="T")
                nc.scalar.activation(t_t[:], s_t[:], AF.Tanh)
                for j in range(G):
                    q = g * G + j
                    for h in range(2):
                        nc.tensor.matmul(
                            scores[:, h * 512 : (h + 1) * 512],
                            bigt[:, P - 1 - q : 2 * P - 1 - q],
                            t_t[:, j, h * 512 : (h + 1) * 512],
                            start=(q == 0),
                            stop=False,
                        )
            # additive mask row broadcast to all query partitions (rank-1)
            for h in range(2):
                nc.tensor.matmul(
                    scores[:, h * 512 : (h + 1) * 512],
                    ones1[:],
                    neg_sb[:, h * 512 : (h + 1) * 512],
                    start=False,
                    stop=True,
                )

            # ---- softmax over v (free dim); scores are bounded by
            # ||attn_v||_1 (~9.2 for this problem scale), so raw exp is safe
            # in fp32 and the max-subtraction can be skipped ----
            expP = const.tile([P, Tv], F32R)
            sumexp_h = const.tile([P, 2], F32)
            for h in range(2):
                nc.scalar.activation(
                    expP[:, h * 512 : (h + 1) * 512],
                    scores[:, h * 512 : (h + 1) * 512],
                    AF.Exp,
                    accum_out=sumexp_h[:, h : h + 1],
                )
            sumexp = const.tile([P, 1], F32)
            nc.vector.tensor_reduce(
                sumexp[:], sumexp_h[:], axis=mybir.AxisListType.X,
                op=mybir.AluOpType.add,
            )
            recip = const.tile([P, 1], F32)
            nc.vector.reciprocal(recip[:], sumexp[:])

            # ---- P^T, final matmul, scale ----
            pT_sb = const.tile([P, VB, P], F32R)
            for half in range(2):
                trp = ps_tr.tile([P, 512], F32R, tag="tr_r")
                for i in range(4):
                    vb = half * 4 + i
                    nc.tensor.transpose(
                        trp[:, i * P : (i + 1) * P],
                        expP[:, vb * P : (vb + 1) * P],
                        ident_r[:],
                    )
                nc.any.tensor_copy(pT_sb[:, half * 4 : (half + 1) * 4, :], trp[:])

            po = ps_proj.tile([P, 512], F32, tag="proj")
            for vb in range(VB):
                nc.tensor.matmul(
                    po[:],
                    pT_sb[:, vb, :],
                    inp_h[vb // 4][:, vb % 4, :],
                    start=(vb == 0),
                    stop=(vb == VB - 1),
                )
            out_sb = const.tile([P, D], F32)
            nc.vector.tensor_scalar_mul(out_sb[:], po[:], recip[:])
            nc.sync.dma_start(y_d.ap(), out_sb[:])

    nc.compile()
    return nc


_NC_CACHE = None


def _get_nc():
    global _NC_CACHE
    if _NC_CACHE is None:
        _NC_CACHE = build_nc()
    return _NC_CACHE


def kernel(inputs, context, mask, Wk, bk, Wq, bq, attn_v):
    nc = _get_nc()
    f32 = np.float32
    in_maps = []
    for c in range(NCORES):
        b, qh = c // 2, c % 2
        wkq = np.concatenate(
            [np.asarray(Wk, dtype=f32), np.asarray(Wq, dtype=f32)], axis=1
        )
        bba = np.stack(
            [np.asarray(bk, f32), np.asarray(bq, f32), np.asarray(attn_v, f32)],
            axis=1,
        )
        in_maps.append({
            "inp": np.ascontiguousarray(inputs[b], dtype=f32),
            "ctx": np.ascontiguousarray(
                context[b, qh * QC : (qh + 1) * QC], dtype=f32
            ),
            "mask": np.ascontiguousarray(mask[b : b + 1, :], dtype=np.int32),
            "wkq": np.ascontiguousarray(wkq),
            "bba": np.ascontiguousarray(bba),
        })
    res = bass_utils.run_bass_kernel_spmd(nc, in_maps, core_ids=list(range(NCORES)))
    out = np.empty((B, Tq, D), f32)
    for c in range(NCORES):
        b, qh = c // 2, c % 2
        out[b, qh * QC : (qh + 1) * QC, :] = res.results[c]["y"]
    return out


# revision 18
# speedup vs baseline: 1.0105x; 1.0037x over previous
# Additive (Bahdanau) attention Trainium2 kernel.
#
# Problem shapes (hardcoded): B=4, Tq=256, Tv=1024, D=512, A=128.
#   k = inputs @ Wk + bk                  [B,Tv,A]
#   q = context @ Wq + bq                 [B,Tq,A]
#   scores[b,i,v] = sum_a attn_v[a] * tanh(q[b,i,a] + k[b,v,a]) + (1-mask)*NEG_BIG
#   out = softmax_v(scores) @ inputs      [B,Tq,D]
#
# Sharding: 8 cores = (batch b = c//2) x (query half qh = c%2); each core owns
# 128 queries with the full Tv, so softmax is local and no collectives are
# needed.
#
# Per-core dataflow (ACT/tanh-bound):
#   PE:  transpose inputs/context -> kT[a,v] (PSUM->SBUF), qT[a,q] projections
#   DVE: S[a, (j,v)] = kT[a,v] + qb[a, q]      (tensor_scalar, 2x mode)
#   ACT: T = tanh(S) on G-query batches        (the 16.8M-element bottleneck)
#   PE:  scores[q,v] accumulated with shifted one-hot weight columns so each
#        query's weighted A-reduction lands on its own PSUM partition
#   softmax: DVE reduce_max(negate) -> ACT exp(bias=-max, accum_out=sumexp)
#   PE:  transpose exp(P) -> P^T; out = P^T.T @ inputs; scale by 1/sumexp

import numpy as np

import concourse.bass as bass
import concourse.tile as tile
from concourse import bacc, mybir
from concourse import bass_utils
from concourse.masks import make_identity

P = 128
B, Tq, Tv, D, A = 4, 256, 1024, 512, 128
NCORES = 8
QC = Tq // 2          # queries per core
DC = D // P           # d chunks (4)
VB = Tv // P          # v blocks (8)
G = 4                 # queries per tanh batch
NG = QC // G          # groups (32)
NEG_BIG = -1e9

F32 = mybir.dt.float32
F32R = mybir.dt.float32r
I32 = mybir.dt.int32
AF = mybir.ActivationFunctionType


def _r(ap):
    # fp32 matmuls stream at 4 cycles/row on the PE; float32r (same bytes,
    # reduced-precision multiply, fp32 accumulate) streams at 1 cycle/row
    # for free dims >= 256.
    return ap.bitcast(F32R)


def build_nc():
    nc = bacc.Bacc("TRN2", target_bir_lowering=False, debug=False)

    inp_d = nc.dram_tensor("inp", (Tv, D), F32R, kind="ExternalInput")
    ctx_d = nc.dram_tensor("ctx", (QC, D), F32R, kind="ExternalInput")
    msk_d = nc.dram_tensor("mask", (1, Tv), I32, kind="ExternalInput")
    wkq_d = nc.dram_tensor("wkq", (D, 2 * A), F32R, kind="ExternalInput")
    bba_d = nc.dram_tensor("bba", (A, 3), F32, kind="ExternalInput")
    y_d = nc.dram_tensor("y", (QC, D), F32, kind="ExternalOutput")

    with tile.TileContext(nc) as tc:
        with (
            tc.tile_pool(name="const", bufs=1) as const,
            tc.tile_pool(name="spool", bufs=3) as spool,
            tc.tile_pool(name="tpool", bufs=3) as tpool,
            tc.tile_pool(name="ps_tr", bufs=2, space="PSUM") as ps_tr,
            tc.tile_pool(name="ps_proj", bufs=2, space="PSUM") as ps_proj,
            tc.tile_pool(name="ps_sc", bufs=1, space="PSUM") as ps_sc,
        ):
            # ---- loads (DMA count minimized: ~650ns HWDGE issue each) ----
            ctx_sb = const.tile([P, D], F32R)
            nc.sync.dma_start(ctx_sb[:], ctx_d.ap())
            wkq_sb = const.tile([P, DC, 2 * A], F32R)
            nc.sync.dma_start(wkq_sb[:], wkq_d.ap().rearrange("(o p) a -> p o a", p=P))
            inp_re = inp_d.ap().rearrange("(o p) d -> p o d", p=P)
            inp_h = [const.tile([P, 4, D], F32R, name=f"inph{h}") for h in range(2)]
            nc.sync.dma_start(inp_h[0][:], inp_re[:, 0:4, :])
            bba_sb = const.tile([P, 3], F32)
            nc.sync.dma_start(bba_sb[:], bba_d.ap())
            msk_sb = const.tile([1, Tv], I32)
            nc.sync.dma_start(msk_sb[:], msk_d.ap())
            nc.sync.dma_start(inp_h[1][:], inp_re[:, 4:8, :])
            bk_sb = bba_sb[:, 0:1]
            bq_sb = bba_sb[:, 1:2]
            av_sb = bba_sb[:, 2:3]
            wk_sb = wkq_sb[:, :, 0:A]
            wq_sb = wkq_sb[:, :, A : 2 * A]

            ident = const.tile([P, P], F32)
            make_identity(nc, ident[:])
            ident_r = const.tile([P, P], F32R)
            nc.vector.tensor_copy(ident_r[:], ident[:])

            # mask -> additive row: neg[v] = mask*1e9 - 1e9  (0 if mask==1)
            mskf_sb = const.tile([1, Tv], F32)
            nc.vector.tensor_copy(mskf_sb[:], msk_sb[:])
            neg_sb = const.tile([1, Tv], F32R)
            nc.vector.tensor_scalar(
                neg_sb[:], mskf_sb[:], -NEG_BIG, NEG_BIG,
                mybir.AluOpType.mult, mybir.AluOpType.add,
            )
            stage = const.tile([P, 2 * P - 1], F32)
            nc.vector.memset(stage[:], 0.0)
            ones1 = const.tile([1, P], F32R)
            onesf = const.tile([1, P], F32)
            nc.vector.memset(onesf[:], 1.0)
            nc.vector.tensor_copy(ones1[:], onesf[:])

            # shifted one-hot weights: BIGT[:, 127] = attn_v, else 0
            bigt = const.tile([P, 2 * P - 1], F32R)
            nc.vector.tensor_copy(bigt[:], stage[:])
            nc.vector.tensor_copy(bigt[:, P - 1 : P], av_sb[:])

            # ---- transposes: context -> ctxT [d, q], inputs -> inputsT [d, v] ----
            ctxT_sb = const.tile([P, DC, P], F32R)
            trc = ps_tr.tile([P, 512], F32R, tag="tr_r")
            for dc in range(DC):
                nc.tensor.transpose(
                    trc[:, dc * P : (dc + 1) * P],
                    ctx_sb[:, dc * P : (dc + 1) * P],
                    ident_r[:],
                )
            nc.vector.tensor_copy(ctxT_sb[:], trc[:])

            inpT_sb = const.tile([P, DC, Tv], F32R)
            for vb in range(VB):
                trv = ps_tr.tile([P, 512], F32R, tag="tr_r")
                for dc in range(DC):
                    nc.tensor.transpose(
                        trv[:, dc * P : (dc + 1) * P],
                        inp_h[vb // 4][:, vb % 4, dc * P : (dc + 1) * P],
                        ident_r[:],
                    )
                nc.scalar.copy(
                    inpT_sb[:, :, vb * P : (vb + 1) * P], trv[:]
                )

            # ---- projections ----
            # kT[a, v] = sum_d Wk[d,a] * inputsT[d,v]
            kT_sb = const.tile([P, Tv], F32)
            for h in range(2):
                pk = ps_proj.tile([P, 512], F32, tag="proj")
                for dc in range(DC):
                    nc.tensor.matmul(
                        pk[:],
                        wk_sb[:, dc, :],
                        inpT_sb[:, dc, h * 512 : (h + 1) * 512],
                        start=(dc == 0),
                        stop=(dc == DC - 1),
                    )
                nc.vector.tensor_copy(kT_sb[:, h * 512 : (h + 1) * 512], pk[:])

            # qb[a, q] = sum_d Wq[d,a] * ctxT[d,q] + (bk+bq)[a]
            bkq_sb = const.tile([P, 1], F32)
            nc.vector.tensor_add(bkq_sb[:], bk_sb[:], bq_sb[:])
            pq = ps_proj.tile([P, P], F32, tag="proj")
            for dc in range(DC):
                nc.tensor.matmul(
                    pq[:],
                    wq_sb[:, dc, :],
                    ctxT_sb[:, dc, :],
                    start=(dc == 0),
                    stop=(dc == DC - 1),
                )
            qb_sb = const.tile([P, P], F32)
            nc.vector.tensor_scalar_add(qb_sb[:], pq[:], bkq_sb[:])

            # ---- main loop: tanh batches + one-hot score reduction ----
            scores = ps_sc.tile([P, Tv], F32)
            # First NPRE groups run h-major: all half-0 work is emitted before
            # any half-1 work, so the in-order DVE/ACT streams never block on
            # the second half of kT (which waits on the second input DMA).
            NPRE = 3
            s_pre = [spool.tile([P, G, Tv], F32, tag="S", name=f"s_pre{i}") for i in range(NPRE)]
            t_pre = [tpool.tile([P, G, Tv], F32R, tag="T", name=f"t_pre{i}") for i in range(NPRE)]
            for h in range(2):
                for g in range(NPRE):
                    for j in range(G):
                        nc.vector.tensor_scalar_add(
                            s_pre[g][:, j, h * 512 : (h + 1) * 512],
                            kT_sb[:, h * 512 : (h + 1) * 512],
                            qb_sb[:, g * G + j : g * G + j + 1],
                        )
                    nc.scalar.activation(
                        t_pre[g][:, :, h * 512 : (h + 1) * 512],
                        s_pre[g][:, :, h * 512 : (h + 1) * 512],
                        AF.Tanh,
                    )
                    for j in range(G):
                        q = g * G + j
                        nc.tensor.matmul(
                            scores[:, h * 512 : (h + 1) * 512],
                            bigt[:, P - 1 - q : 2 * P - 1 - q],
                            t_pre[g][:, j, h * 512 : (h + 1) * 512],
                            start=(q == 0),
                            stop=False,
                        )
            for g in range(NPRE, NG):
                s_t = spool.tile([P, G, Tv], F32, tag="S")
                for j in range(G):
                    nc.vector.tensor_scalar_add(
                        s_t[:, j, :], kT_sb[:], qb_sb[:, g * G + j : g * G + j + 1]
                    )
                t_t = tpool.tile([P, G, Tv], F32R, tag="T")
                if g == NG - 1:
                    # h-major tail: PE h0 matmuls overlap the h1 tanh, so the
                    # softmax/output chain starts sooner
                    for h in range(2):
                        nc.scalar.activation(
                            t_t[:, :, h * 512 : (h + 1) * 512],
                            s_t[:, :, h * 512 : (h + 1) * 512],
                            AF.Tanh,
                        )
                        for j in range(G):
                            q = g * G + j
                            nc.tensor.matmul(
                                scores[:, h * 512 : (h + 1) * 512],
                                bigt[:, P - 1 - q : 2 * P - 1 - q],
                                t_t[:, j, h * 512 : (h + 1) * 512],
                                start=False,
                                stop=False,
                            )
                    continue
                nc.scalar.activation(t_t[:], s_t[:], AF.Tanh)
                for j in range(G):
                    q = g * G + j
                    for h in range(2):
                        nc.tensor.matmul(
                            scores[:, h * 512 : (h + 1) * 512],
                            bigt[:, P - 1 - q : 2 * P - 1 - q],
                            t_t[:, j, h * 512 : (h + 1) * 512],
                            start=(q == 0),
                            stop=False,
                        )
            # additive mask row broadcast to all query partitions (rank-1)
            for h in range(2):
                nc.tensor.matmul(
                    scores[:, h * 512 : (h + 1) * 512],
                    ones1[:],
                    neg_sb[:, h * 512 : (h + 1) * 512],
                    start=False,
                    stop=True,
                )

            # ---- softmax over v (free dim); scores are bounded by
            # ||attn_v||_1 (~9.2 for this problem scale), so raw exp is safe
            # in fp32 and the max-subtraction can be skipped ----
            expP = const.tile([P, Tv], F32R)
            sumexp_h = const.tile([P, 2], F32)
            for h in range(2):
                nc.scalar.activation(
                    expP[:, h * 512 : (h + 1) * 512],
                    scores[:, h * 512 : (h + 1) * 512],
                    AF.Exp,
                    accum_out=sumexp_h[:, h : h + 1],
                )
            sumexp = const.tile([P, 1], F32)
            nc.vector.tensor_reduce(
                sumexp[:], sumexp_h[:], axis=mybir.AxisListType.X,
                op=mybir.AluOpType.add,
            )
            recip = const.tile([P, 1], F32)
            nc.vector.reciprocal(recip[:], sumexp[:])

            # ---- P^T, final matmul, scale ----
            pT_sb = const.tile([P, VB, P], F32R)
            for half in range(2):
                trp = ps_tr.tile([P, 512], F32R, tag="tr_r")
                for i in range(4):
                    vb = half * 4 + i
                    nc.tensor.transpose(
                        trp[:, i * P : (i + 1) * P],
                        expP[:, vb * P : (vb + 1) * P],
                        ident_r[:],
                    )
                nc.scalar.copy(pT_sb[:, half * 4 : (half + 1) * 4, :], trp[:])

            po = ps_proj.tile([P, 512], F32, tag="proj")
            for vb in range(VB):
                nc.tensor.matmul(
                    po[:],
                    pT_sb[:, vb, :],
                    inp_h[vb // 4][:, vb % 4, :],
                    start=(vb == 0),
                    stop=(vb == VB - 1),
                )
            out_sb = const.tile([P, D], F32)
            nc.vector.tensor_scalar_mul(out_sb[:], po[:], recip[:])
            nc.sync.dma_start(y_d.ap(), out_sb[:])

    nc.compile()
    return nc


_NC_CACHE = None


def _get_nc():
    global _NC_CACHE
    if _NC_CACHE is None:
        _NC_CACHE = build_nc()
    return _NC_CACHE


def kernel(inputs, context, mask, Wk, bk, Wq, bq, attn_v):
    nc = _get_nc()
    f32 = np.float32
    in_maps = []
    for c in range(NCORES):
        b, qh = c // 2, c % 2
        wkq = np.concatenate(
            [np.asarray(Wk, dtype=f32), np.asarray(Wq, dtype=f32)], axis=1
        )
        bba = np.stack(
            [np.asarray(bk, f32), np.asarray(bq, f32), np.asarray(attn_v, f32)],
            axis=1,
        )
        in_maps.append({
            "inp": np.ascontiguousarray(inputs[b], dtype=f32),
            "ctx": np.ascontiguousarray(
                context[b, qh * QC : (qh + 1) * QC], dtype=f32
            ),
            "mask": np.ascontiguousarray(mask[b : b + 1, :], dtype=np.int32),
            "wkq": np.ascontiguousarray(wkq),
            "bba": np.ascontiguousarray(bba),
        })
    res = bass_utils.run_bass_kernel_spmd(nc, in_maps, core_ids=list(range(NCORES)))
    out = np.empty((B, Tq, D), f32)
    for c in range(NCORES):
        b, qh = c // 2, c % 2
        out[b, qh * QC : (qh + 1) * QC, :] = res.results[c]["y"]
    return out


# revision 19
# speedup vs baseline: 1.0397x; 1.0289x over previous
# Additive (Bahdanau) attention Trainium2 kernel.
#
# Problem shapes (hardcoded): B=4, Tq=256, Tv=1024, D=512, A=128.
#   k = inputs @ Wk + bk                  [B,Tv,A]
#   q = context @ Wq + bq                 [B,Tq,A]
#   scores[b,i,v] = sum_a attn_v[a] * tanh(q[b,i,a] + k[b,v,a]) + (1-mask)*NEG_BIG
#   out = softmax_v(scores) @ inputs      [B,Tq,D]
#
# Sharding: 8 cores = (batch b = c//2) x (query half qh = c%2); each core owns
# 128 queries with the full Tv, so softmax is local and no collectives are
# needed.
#
# Per-core dataflow (ACT/tanh-bound; other engines hide under it):
#   PE:  transpose inputs/context -> kT[a,v], qb[a,q] projections (float32r)
#   DVE: S[a, (j,v)] = kT[a,v] + qb[a,q]       (tensor_scalar, 2x mode)
#   ACT: T = tanh(S) on G-query batches        (the 16.8M-element bottleneck)
#   PE:  scores[q,v] accumulated with shifted one-hot weight columns so each
#        query's weighted A-reduction lands on its own PSUM partition
#   softmax: raw exp (scores bounded by ||attn_v||_1 ~ 9.2) + accum_out sum
#   PE:  transpose exp(P) -> P^T; out = P^T.T @ inputs; scale by 1/sumexp
#
# The engines execute their instruction streams in order, so emission order
# below is hand-interleaved: input halves flow DMA -> PE transpose -> evac
# (alternating ACT/DVE) -> kproj -> preadds so the ACT tanh stream starts as
# early as possible and never stalls.

import numpy as np

import concourse.bass as bass
import concourse.tile as tile
from concourse import bacc, mybir
from concourse import bass_utils
from concourse.masks import make_identity

P = 128
B, Tq, Tv, D, A = 4, 256, 1024, 512, 128
NCORES = 8
QC = Tq // 2          # queries per core
DC = D // P           # d chunks (4)
VB = Tv // P          # v blocks (8)
G = 4                 # queries per tanh batch
NG = QC // G          # groups (32)
NPRE = 3              # pipeline-prefill groups, emitted per-half
NEG_BIG = -1e9

F32 = mybir.dt.float32
F32R = mybir.dt.float32r
I32 = mybir.dt.int32
AF = mybir.ActivationFunctionType


def build_nc():
    nc = bacc.Bacc("TRN2", target_bir_lowering=False, debug=False)

    inp_d = nc.dram_tensor("inp", (Tv, D), F32R, kind="ExternalInput")
    ctx_d = nc.dram_tensor("ctx", (QC, D), F32R, kind="ExternalInput")
    msk_d = nc.dram_tensor("mask", (1, Tv), I32, kind="ExternalInput")
    wkq_d = nc.dram_tensor("wkq", (D, 2 * A), F32R, kind="ExternalInput")
    bba_d = nc.dram_tensor("bba", (A, 3), F32, kind="ExternalInput")
    y_d = nc.dram_tensor("y", (QC, D), F32, kind="ExternalOutput")

    with tile.TileContext(nc) as tc:
        with (
            tc.tile_pool(name="const", bufs=1) as const,
            tc.tile_pool(name="spool", bufs=3) as spool,
            tc.tile_pool(name="tpool", bufs=3) as tpool,
            tc.tile_pool(name="ps_tr", bufs=2, space="PSUM") as ps_tr,
            tc.tile_pool(name="ps_proj", bufs=2, space="PSUM") as ps_proj,
            tc.tile_pool(name="ps_sc", bufs=1, space="PSUM") as ps_sc,
        ):
            # ---- loads (DMA issue overhead ~650ns each; count minimized,
            # ordered so the first input half lands as early as possible) ----
            ctx_sb = const.tile([P, D], F32R)
            nc.sync.dma_start(ctx_sb[:], ctx_d.ap())
            wkq_sb = const.tile([P, DC, 2 * A], F32R)
            nc.sync.dma_start(wkq_sb[:], wkq_d.ap().rearrange("(o p) a -> p o a", p=P))
            inp_re = inp_d.ap().rearrange("(o p) d -> p o d", p=P)
            inp_h = [const.tile([P, 4, D], F32R, name=f"inph{h}") for h in range(2)]
            nc.sync.dma_start(inp_h[0][:], inp_re[:, 0:4, :])
            bba_sb = const.tile([P, 3], F32)
            nc.sync.dma_start(bba_sb[:], bba_d.ap())
            msk_sb = const.tile([1, Tv], I32)
            nc.sync.dma_start(msk_sb[:], msk_d.ap())
            nc.sync.dma_start(inp_h[1][:], inp_re[:, 4:8, :])
            bk_sb = bba_sb[:, 0:1]
            bq_sb = bba_sb[:, 1:2]
            av_sb = bba_sb[:, 2:3]
            wk_sb = wkq_sb[:, :, 0:A]
            wq_sb = wkq_sb[:, :, A : 2 * A]

            # ---- small constants ----
            ident = const.tile([P, P], F32)
            make_identity(nc, ident[:])
            ident_r = const.tile([P, P], F32R)
            nc.vector.tensor_copy(ident_r[:], ident[:])

            # mask -> additive row: neg[v] = mask*1e9 - 1e9  (0 if mask==1)
            mskf_sb = const.tile([1, Tv], F32)
            nc.vector.tensor_copy(mskf_sb[:], msk_sb[:])
            neg_sb = const.tile([1, Tv], F32R)
            nc.vector.tensor_scalar(
                neg_sb[:], mskf_sb[:], -NEG_BIG, NEG_BIG,
                mybir.AluOpType.mult, mybir.AluOpType.add,
            )
            stage = const.tile([P, 2 * P - 1], F32)
            nc.gpsimd.memset(stage[:], 0.0)
            ones1 = const.tile([1, P], F32R)
            nc.vector.tensor_copy(ones1[:], stage[0:1, 0:P])
            nc.vector.tensor_scalar_add(ones1[:], ones1[:], 1.0)

            # shifted one-hot weights: BIGT[:, 127] = attn_v, else 0
            bigt = const.tile([P, 2 * P - 1], F32R)
            nc.vector.tensor_copy(bigt[:], stage[:])
            nc.vector.tensor_copy(bigt[:, P - 1 : P], av_sb[:])

            # ---- context transposes + q projection (early; only needs ctx) ----
            ctxT_sb = const.tile([P, DC, P], F32R)
            trc = ps_tr.tile([P, 512], F32R, tag="tr_r")
            for dc in range(DC):
                nc.tensor.transpose(
                    trc[:, dc * P : (dc + 1) * P],
                    ctx_sb[:, dc * P : (dc + 1) * P],
                    ident_r[:],
                )
            nc.vector.tensor_copy(ctxT_sb[:], trc[:])

            bkq_sb = const.tile([P, 1], F32)
            nc.vector.tensor_add(bkq_sb[:], bk_sb[:], bq_sb[:])
            pq = ps_proj.tile([P, P], F32, tag="proj")
            for dc in range(DC):
                nc.tensor.matmul(
                    pq[:],
                    wq_sb[:, dc, :],
                    ctxT_sb[:, dc, :],
                    start=(dc == 0),
                    stop=(dc == DC - 1),
                )
            qb_sb = const.tile([P, P], F32)
            nc.vector.tensor_scalar_add(qb_sb[:], pq[:], bkq_sb[:])

            # ---- per-half input pipeline + prefill groups (h-major) ----
            inpT_sb = const.tile([P, DC, Tv], F32R)
            kT_sb = const.tile([P, Tv], F32)
            scores = ps_sc.tile([P, Tv], F32)
            s_pre = [
                spool.tile([P, G, Tv], F32, tag="S", name=f"s_pre{i}")
                for i in range(NPRE)
            ]
            t_pre = [
                tpool.tile([P, G, Tv], F32R, tag="T", name=f"t_pre{i}")
                for i in range(NPRE)
            ]

            def emit_half(h):
                # transposes for this half's 4 v-blocks; evacs alternate
                # ACT/DVE so neither in-order stream serializes the chain
                for i in range(4):
                    vb = h * 4 + i
                    trv = ps_tr.tile([P, 512], F32R, tag="tr_r")
                    for dc in range(DC):
                        nc.tensor.transpose(
                            trv[:, dc * P : (dc + 1) * P],
                            inp_h[h][:, i, dc * P : (dc + 1) * P],
                            ident_r[:],
                        )
                    dst = inpT_sb[:, :, vb * P : (vb + 1) * P]
                    if i % 2 == 0:
                        nc.scalar.copy(dst, trv[:])
                    else:
                        nc.vector.tensor_copy(dst, trv[:])
                # k projection for this half
                pk = ps_proj.tile([P, 512], F32, tag="proj")
                for dc in range(DC):
                    nc.tensor.matmul(
                        pk[:],
                        wk_sb[:, dc, :],
                        inpT_sb[:, dc, h * 512 : (h + 1) * 512],
                        start=(dc == 0),
                        stop=(dc == DC - 1),
                    )
                nc.vector.tensor_copy(kT_sb[:, h * 512 : (h + 1) * 512], pk[:])

            def pre_tanh_mm(g, h):
                for j in range(G):
                    nc.vector.tensor_scalar_add(
                        s_pre[g][:, j, h * 512 : (h + 1) * 512],
                        kT_sb[:, h * 512 : (h + 1) * 512],
                        qb_sb[:, g * G + j : g * G + j + 1],
                    )
                nc.scalar.activation(
                    t_pre[g][:, :, h * 512 : (h + 1) * 512],
                    s_pre[g][:, :, h * 512 : (h + 1) * 512],
                    AF.Tanh,
                )
                for j in range(G):
                    q = g * G + j
                    nc.tensor.matmul(
                        scores[:, h * 512 : (h + 1) * 512],
                        bigt[:, P - 1 - q : 2 * P - 1 - q],
                        t_pre[g][:, j, h * 512 : (h + 1) * 512],
                        start=(q == 0),
                        stop=False,
                    )

            emit_half(0)
            pre_tanh_mm(0, 0)      # first tanh as soon as kT half 0 exists
            emit_half(1)           # second input half flows while tanh runs
            pre_tanh_mm(1, 0)
            pre_tanh_mm(2, 0)
            for g in range(NPRE):
                pre_tanh_mm(g, 1)

            # ---- steady-state groups ----
            for g in range(NPRE, NG - 1):
                s_t = spool.tile([P, G, Tv], F32, tag="S")
                for j in range(G):
                    nc.vector.tensor_scalar_add(
                        s_t[:, j, :], kT_sb[:], qb_sb[:, g * G + j : g * G + j + 1]
                    )
                t_t = tpool.tile([P, G, Tv], F32R, tag="T")
                nc.scalar.activation(t_t[:], s_t[:], AF.Tanh)
                for j in range(G):
                    q = g * G + j
                    for h in range(2):
                        nc.tensor.matmul(
                            scores[:, h * 512 : (h + 1) * 512],
                            bigt[:, P - 1 - q : 2 * P - 1 - q],
                            t_t[:, j, h * 512 : (h + 1) * 512],
                            start=(q == 0),
                            stop=False,
                        )

            # ---- last group, h-major, so the h0 softmax/output chain
            # overlaps the h1 tanh; masks interleaved to close each half ----
            gl = NG - 1
            s_l = spool.tile([P, G, Tv], F32, tag="S")
            for j in range(G):
                nc.vector.tensor_scalar_add(
                    s_l[:, j, :], kT_sb[:], qb_sb[:, gl * G + j : gl * G + j + 1]
                )
            t_l = tpool.tile([P, G, Tv], F32R, tag="T")
            for h in range(2):
                nc.scalar.activation(
                    t_l[:, :, h * 512 : (h + 1) * 512],
                    s_l[:, :, h * 512 : (h + 1) * 512],
                    AF.Tanh,
                )
                for j in range(G):
                    q = gl * G + j
                    nc.tensor.matmul(
                        scores[:, h * 512 : (h + 1) * 512],
                        bigt[:, P - 1 - q : 2 * P - 1 - q],
                        t_l[:, j, h * 512 : (h + 1) * 512],
                        start=False,
                        stop=False,
                    )
                # additive mask row for this half (rank-1 broadcast), closes
                # the accumulation group so exp can start
                nc.tensor.matmul(
                    scores[:, h * 512 : (h + 1) * 512],
                    ones1[:],
                    neg_sb[:, h * 512 : (h + 1) * 512],
                    start=False,
                    stop=True,
                )

            # ---- softmax over v; raw exp is safe: |scores| <= ||attn_v||_1 ----
            expP = const.tile([P, Tv], F32R)
            sumexp_h = const.tile([P, 2], F32)
            for h in range(2):
                nc.scalar.activation(
                    expP[:, h * 512 : (h + 1) * 512],
                    scores[:, h * 512 : (h + 1) * 512],
                    AF.Exp,
                    accum_out=sumexp_h[:, h : h + 1],
                )
            sumexp = const.tile([P, 1], F32)
            nc.vector.tensor_reduce(
                sumexp[:], sumexp_h[:], axis=mybir.AxisListType.X,
                op=mybir.AluOpType.add,
            )
            recip = const.tile([P, 1], F32)
            nc.vector.reciprocal(recip[:], sumexp[:])

            # ---- P^T (per half), final matmul, scale ----
            pT_sb = const.tile([P, VB, P], F32R)
            po = ps_proj.tile([P, 512], F32, tag="proj")
            for half in range(2):
                trp = ps_tr.tile([P, 512], F32R, tag="tr_r")
                for i in range(4):
                    vb = half * 4 + i
                    nc.tensor.transpose(
                        trp[:, i * P : (i + 1) * P],
                        expP[:, vb * P : (vb + 1) * P],
                        ident_r[:],
                    )
                nc.vector.tensor_copy(pT_sb[:, half * 4 : (half + 1) * 4, :], trp[:])
                for i in range(4):
                    vb = half * 4 + i
                    nc.tensor.matmul(
                        po[:],
                        pT_sb[:, vb, :],
                        inp_h[half][:, i, :],
                        start=(vb == 0),
                        stop=(vb == VB - 1),
                    )
            out_sb = const.tile([P, D], F32)
            nc.vector.tensor_scalar_mul(out_sb[:], po[:], recip[:])
            nc.sync.dma_start(y_d.ap(), out_sb[:])

    nc.compile()
    return nc


_NC_CACHE = None


def _get_nc():
    global _NC_CACHE
    if _NC_CACHE is None:
        _NC_CACHE = build_nc()
    return _NC_CACHE


def kernel(inputs, context, mask, Wk, bk, Wq, bq, attn_v):
    nc = _get_nc()
    f32 = np.float32
    wkq = np.concatenate(
        [np.asarray(Wk, dtype=f32), np.asarray(Wq, dtype=f32)], axis=1
    )
    bba = np.stack(
        [np.asarray(bk, f32), np.asarray(bq, f32), np.asarray(attn_v, f32)],
        axis=1,
    )
    in_maps = []
    for c in range(NCORES):
        b, qh = c // 2, c % 2
        in_maps.append({
            "inp": np.ascontiguousarray(inputs[b], dtype=f32),
            "ctx": np.ascontiguousarray(
                context[b, qh * QC : (qh + 1) * QC], dtype=f32
            ),
            "mask": np.ascontiguousarray(mask[b : b + 1, :], dtype=np.int32),
            "wkq": np.ascontiguousarray(wkq),
            "bba": np.ascontiguousarray(bba),
        })
    res = bass_utils.run_bass_kernel_spmd(nc, in_maps, core_ids=list(range(NCORES)))
    out = np.empty((B, Tq, D), f32)
    for c in range(NCORES):
        b, qh = c // 2, c % 2
        out[b, qh * QC : (qh + 1) * QC, :] = res.results[c]["y"]
    return out
